# revision 1
# baseline (speedup 1.0000x reference)
"""Bass/Trainium2 kernel for nn_DSQGAttentionD41J16D (sparse offset attention).

Sharding: 16 heads over 8 cores -> 2 heads per core (SPMD). Host lays inputs
out transposed per core as [dh = h*64+d (128 partitions), 1024-pad + n] bf16,
so every offset-shift k[n-d_i] / v[n-d_i] is a free-dim slice on chip.

Per chunk of 1024 query positions (4 chunks):
  scores:  prod_i = qT * kT[:, n-d_i]            (DVE bf16 2x, odd offsets
           read directly at 2-byte alignment)
           pss[(i,h), n] = se-matmul + selector-matmuls partition-reduce (PE)
           + -1e30 validity mask; p = exp(0.125*pss + pos_bias) on ACT
           (pos_bias rides the activation bias port)
  denom:   psl8[(hf,h), n] = ones-matmul row sums (PE); shipped to host as l
           (no on-chip reciprocal/normalize -- host divides)
  PV:      p rows broadcast across the 64 d-partitions of each head by a
           repeat-read DMA; tmp_i = p_bc * vT[:, n-d_i] (split DVE/GPSIMD);
           acc accumulated over offsets on PE (identity matmuls, PSUM fp32)
  out:     ACT copies acc PSUM -> SBUF bf16; DMA out; host untransposes,
           divides by l, zeroes the n=0 row.
"""

import os
import sys

sys.path.insert(0, "/opt/trn_rl_repo")

import numpy as np
import ml_dtypes

ALL_OFFSETS = [1, 3, 4, 13, 15, 21, 23, 28, 48, 64, 96, 192, 384, 512, 768, 1024]
N = 4096
HD = 64
P = 128
PAD = 1024
NT = PAD + N
NOFF = 16
C = 1024          # chunk width
NCH = N // C      # 4 chunks
NEG = -1.0e30

BF16 = ml_dtypes.bfloat16

# PV multiply jobs routed to the GPSIMD (Pool) engine: set of (chunk, offset).
# Pool is idle early, so it takes more of chunks 0/1 (whose p-broadcasts land
# while the DVE is still busy with score products).
POOL_PV = (
    {(0, i) for i in (1, 3, 5, 7, 9, 11)}
    | {(1, i) for i in (1, 3, 5, 7, 9, 11)}
    | {(2, i) for i in (1, 4, 7, 10, 13)}
    | {(3, i) for i in (2, 5, 8, 11, 14)}
)

_CACHE = {}
TRACE = os.environ.get("BASS_KERNEL_TRACE", "0") == "1"
LAST_RESULTS = [None]

# const blob column layout
SEL0 = 0            # sel: 16 offsets x 32 cols
SET0 = 512          # seT: 32 cols
ID0 = 544           # ident: 128 cols
MKT0 = 672          # maskT: 32 cols (rows 0:16)
MK0 = 704           # mask01: 1024 cols (rows 0:16)
ONL0 = 1728         # onesl: 4 x 8 cols (rows 0:32)
BLOBW = 1760


def _build_blob(scale_embed_np):
    blob = np.zeros((P, BLOBW), dtype=BF16)
    for h in range(2):
        for d in range(HD):
            p = 64 * h + d
            for i in range(NOFF):
                blob[p, SEL0 + 32 * i + 2 * i + h] = 1.0
                blob[p, SET0 + 2 * i + h] = BF16(scale_embed_np[i, d])
    blob[:, ID0 : ID0 + P] = np.eye(P, dtype=BF16)
    for j in range(NOFF):
        blob[j, MKT0 + 2 * j] = NEG
        blob[j, MKT0 + 2 * j + 1] = NEG
        blob[j, MK0 : MK0 + ALL_OFFSETS[j]] = 1.0
    for hf in range(4):
        for i in range(NOFF):
            for h in range(2):
                blob[2 * i + h, ONL0 + 8 * hf + 2 * hf + h] = 1.0
    return blob


def _build(scale_embed_np):
    import concourse.bass as bass
    import concourse.mybir as mybir
    import concourse.tile as tile
    from concourse import bacc

    fp32 = mybir.dt.float32
    bf16 = mybir.dt.bfloat16
    MULT = mybir.AluOpType.mult
    EXP = mybir.ActivationFunctionType.Exp
    COPY = mybir.ActivationFunctionType.Copy

    nc = bacc.Bacc()

    qT_in = nc.dram_tensor("qT_in", [P, NT], bf16, kind="ExternalInput")
    kT_in = nc.dram_tensor("kT_in", [P, NT], bf16, kind="ExternalInput")
    vT_in = nc.dram_tensor("vT_in", [P, NT], bf16, kind="ExternalInput")
    pb_in = nc.dram_tensor("pb_in", [2 * NOFF, 1], bf16, kind="ExternalInput")
    oT_out = nc.dram_tensor("oT_out", [P, N], bf16, kind="ExternalOutput")
    l_out = nc.dram_tensor("l_out", [8, 1024], fp32, kind="ExternalOutput")

    blob_c = nc.inline_tensor(_build_blob(scale_embed_np), name="blob_c")

    with tile.TileContext(nc) as tc:
        consts = tc.alloc_tile_pool(name="consts", bufs=1)
        big = tc.alloc_tile_pool(name="big", bufs=1)
        ps_s = tc.alloc_tile_pool(name="ps_s", bufs=2, space="PSUM")
        ps_l = tc.alloc_tile_pool(name="ps_l", bufs=2, space="PSUM")
        ps_a = tc.alloc_tile_pool(name="ps_a", bufs=4, space="PSUM")
        work = tc.alloc_tile_pool(name="work", bufs=4)
        tmps = tc.alloc_tile_pool(name="tmps", bufs=4)
        # every p-broadcast tile of the run can be live before the PV phase
        # drains them; the pool must cover that, or broadcast DMAs stall.
        bcast = tc.alloc_tile_pool(name="bcast", bufs=48)

        qT = big.tile([P, NT], bf16)
        kT = big.tile([P, NT], bf16)
        vT = big.tile([P, NT], bf16)
        p_sb = big.tile([2 * NOFF, N], bf16)
        out_sb = big.tile([P, N], bf16)
        l_sb = big.tile([8, 1024], fp32)
        blob = consts.tile([P, BLOBW], bf16)
        pb_sb = consts.tile([2 * NOFF, 1], bf16)

        # sync queue: loads then the p-broadcast spine (the serial DMA-issue
        # cost rides under the broadcast transfer time). scalar/ACT queue:
        # activations plus a handful of loads/stores only — a data-dependent
        # DMA there would block exps behind it on the ACT sequencer.
        # ---- piece-0 loads ahead of the blob, so first products start early
        nc.sync.dma_start(out=kT[:, PAD : PAD + C], in_=kT_in[:, PAD : PAD + C])
        nc.scalar.dma_start(out=qT[:, PAD : PAD + C], in_=qT_in[:, PAD : PAD + C])
        nc.sync.dma_start(out=blob, in_=blob_c[:, :])
        nc.scalar.dma_start(out=pb_sb, in_=pb_in[:, :])
        nc.gpsimd.memset(kT[:, 0:PAD], 0.0)
        nc.gpsimd.memset(vT[:, 0:PAD], 0.0)
        nc.scalar.dma_start(out=qT[:, PAD + C : NT], in_=qT_in[:, PAD + C : NT])
        nc.sync.dma_start(out=kT[:, PAD + C : NT], in_=kT_in[:, PAD + C : NT])
        nc.scalar.dma_start(out=vT[:, PAD : NT], in_=vT_in[:, PAD : NT])

        # ---- blob slices ----
        def sel_w(i):
            return blob[:, SEL0 + 32 * i : SEL0 + 32 * i + 32]

        seT_w = blob[:, SET0 : SET0 + 32]
        ident_w = blob[:, ID0 : ID0 + P]
        maskT_w = blob[0:NOFF, MKT0 : MKT0 + 32]

        def mask01_x(g):
            return blob[0:NOFF, MK0 + 512 * g : MK0 + 512 * g + 512]

        def onesl_w(hf):
            return blob[0 : 2 * NOFF, ONL0 + 8 * hf : ONL0 + 8 * hf + 8]

        # single warm-up matmul: consumes the blob DMA semaphore wait early
        warm = ps_s.tile([P, 2], fp32, tag="pss", name="warm")
        nc.tensor.matmul(warm[0:32, 0:2], sel_w(0), ident_w[:, 0:2],
                         start=True, stop=True)

        # validity skip rule: offset i fully invalid in 512-chunk g iff
        # delta_i >= 512*(g+1)
        def sel_skip(g, i):
            return ALL_OFFSETS[i] >= 512 * (g + 1)

        psl8 = [None, None]

        def emit_psl(c):
            """Denominator row-sum matmuls for chunk c's two 512-chunks;
            deferred so the PE meets them well after the exps have run."""
            pair = c // 2
            for gl in range(2):
                g = 2 * c + gl
                s0 = 512 * g
                hf = g - 4 * pair
                if hf == 0:
                    psl8[pair] = ps_l.tile(
                        [8, 512], fp32, tag="psl", name=f"psl_{pair}"
                    )
                nc.tensor.matmul(
                    psl8[pair], onesl_w(hf), p_sb[:, s0 : s0 + 512],
                    start=(hf == 0), stop=(hf == 3), skip_group_check=True,
                )

        def emit_scores(c):
            b = C * c
            g0, g1 = 2 * c, 2 * c + 1
            pss = {}
            for gl, g in ((0, g0), (1, g1)):
                pss[gl] = ps_s.tile(
                    [2 * NOFF, 512], fp32, tag="pss", name=f"pss_{g}"
                )
                nc.tensor.matmul(
                    pss[gl], seT_w,
                    qT[:, PAD + 512 * g : PAD + 512 * g + 512],
                    start=True, stop=False, skip_group_check=True,
                )
            valid0 = [i for i in range(NOFF) if not sel_skip(g0, i)]
            valid1 = [i for i in range(NOFF) if not sel_skip(g1, i)]
            masked = g0 < 2
            for i in range(NOFF):
                in0, in1 = i in valid0, i in valid1
                if not (in0 or in1):
                    continue
                d = ALL_OFFSETS[i]
                pr = work.tile([P, C], bf16, tag="prod", name=f"prod_{c}_{i}")
                nc.vector.tensor_tensor(
                    out=pr,
                    in0=qT[:, PAD + b : PAD + b + C],
                    in1=kT[:, PAD + b - d : PAD + b - d + C],
                    op=MULT,
                )
                if in0:
                    nc.tensor.matmul(
                        pss[0], sel_w(i), pr[:, 0:512],
                        start=False,
                        stop=(not masked) and i == valid0[-1],
                        skip_group_check=True,
                    )
                if in1:
                    nc.tensor.matmul(
                        pss[1], sel_w(i), pr[:, 512:1024],
                        start=False,
                        stop=(not masked) and i == valid1[-1],
                        skip_group_check=True,
                    )
                if i == 3 and c > 0:
                    emit_psl(c - 1)
            if masked:
                for gl, g in ((0, g0), (1, g1)):
                    nc.tensor.matmul(
                        pss[gl], maskT_w, mask01_x(g),
                        start=False, stop=True, skip_group_check=True,
                    )
            for gl, g in ((0, g0), (1, g1)):
                nc.scalar.activation(
                    out=p_sb[:, 512 * g : 512 * g + 512], in_=pss[gl],
                    func=EXP, scale=0.125, bias=pb_sb[:, 0:1],
                )

        def vstart(c, i):
            """512-aligned start of the valid region of (chunk c, offset i);
            >= C means fully skippable."""
            v = max(0, ALL_OFFSETS[i] - C * c)
            return (v // 512) * 512

        def emit_pbc(c):
            b = C * c
            # DVE-destined broadcasts first: the DVE reaches this chunk's PV
            # multiplies right after the next chunk's products, while the Pool
            # drains its share later.
            order = [i for i in range(NOFF) if (c, i) not in POOL_PV] + [
                i for i in range(NOFF) if (c, i) in POOL_PV
            ]
            for i in order:
                v5 = vstart(c, i)
                if v5 >= C:
                    continue
                w = C - v5
                rows = p_sb[2 * i : 2 * i + 2, b + v5 : b + v5 + w]
                dst = bcast.tile([P, w], bf16, tag="pbc", name=f"pbc_{c}_{i}")
                rep = bass.AP(
                    tensor=rows.tensor,
                    offset=rows.offset,
                    ap=[list(rows.ap[0]), [0, HD], [1, w]],
                )
                nc.sync.dma_start(out=dst, in_=rep)
                pbc_tiles[(c, i)] = dst

        pbc_tiles = {}
        acc = {}

        def emit_pv(c):
            b = C * c
            alive = [i for i in range(NOFF) if vstart(c, i) < C]
            valid_g = [
                [i for i in alive if vstart(c, i) <= 512 * gl] for gl in range(2)
            ]
            for gl in range(2):
                acc[2 * c + gl] = ps_a.tile(
                    [P, 512], fp32, tag="acc", name=f"acc_{2 * c + gl}"
                )
            for i in alive:
                v5 = vstart(c, i)
                d = ALL_OFFSETS[i]
                w = C - v5
                tmp = tmps.tile([P, w], bf16, tag="tmp", name=f"tmp_{c}_{i}")
                eng = nc.gpsimd if (c, i) in POOL_PV else nc.vector
                eng.tensor_tensor(
                    out=tmp,
                    in0=pbc_tiles[(c, i)],
                    in1=vT[:, PAD + b + v5 - d : PAD + b + v5 - d + w],
                    op=MULT,
                )
                for gl in range(2):
                    if i not in valid_g[gl]:
                        continue
                    s0l = 512 * gl
                    nc.tensor.matmul(
                        acc[2 * c + gl], ident_w,
                        tmp[:, s0l - v5 : s0l - v5 + 512],
                        start=(i == valid_g[gl][0]),
                        stop=(i == valid_g[gl][-1]),
                        skip_group_check=True,
                    )

        def emit_out(c):
            for gl in range(2):
                g = 2 * c + gl
                s0 = 512 * g
                nc.scalar.activation(
                    out=out_sb[:, s0 : s0 + 512], in_=acc[g], func=COPY
                )
            nc.scalar.dma_start(
                out=oT_out[:, C * c : C * c + C], in_=out_sb[:, C * c : C * c + C]
            )

        # ---- software-pipelined emission: pv(c-1) rides between scores(c)
        # and scores(c+1) on every engine queue ----
        for c in range(NCH):
            emit_scores(c)
            emit_pbc(c)
            if c > 0:
                emit_pv(c - 1)
                emit_out(c - 1)
        emit_psl(NCH - 1)
        emit_pv(NCH - 1)
        emit_out(NCH - 1)
        for pair in range(2):
            nc.scalar.activation(
                out=l_sb[:, 512 * pair : 512 * pair + 512],
                in_=psl8[pair], func=COPY,
            )
        nc.scalar.dma_start(out=l_out[:, :], in_=l_sb)

        bcast.release()
        tmps.release()
        work.release()
        ps_a.release()
        ps_l.release()
        ps_s.release()
        big.release()
        consts.release()

    nc.compile()
    return nc


def _prep_inputs(q, k, v, pos_bias):
    """Host-side sharding + layout prep: per core, heads (2c, 2c+1) packed as
    128 partitions (h*64+d), transposed to [dh, pad+n] bf16."""
    def to_T(x):
        xt = np.ascontiguousarray(x[0].transpose(0, 2, 1)).astype(BF16)
        xt = xt.reshape(8, P, N)
        return np.concatenate([np.zeros((8, P, PAD), dtype=BF16), xt], axis=2)

    qT = to_T(q)
    kT = to_T(k)
    vT = to_T(v)

    in_maps = []
    for c in range(8):
        pb = np.zeros((2 * NOFF, 1), dtype=np.float32)
        for i in range(NOFF):
            for hh in range(2):
                pb[2 * i + hh, 0] = pos_bias[i, 2 * c + hh]
        in_maps.append(
            {
                "qT_in": qT[c],
                "kT_in": kT[c],
                "vT_in": vT[c],
                "pb_in": pb.astype(BF16),
            }
        )
    return in_maps


def kernel(q, k, v, pos_bias, scale_embed):
    from concourse.bass_utils import run_bass_kernel_spmd

    q = np.asarray(q)
    k = np.asarray(k)
    v = np.asarray(v)
    pos_bias = np.asarray(pos_bias)
    scale_embed = np.asarray(scale_embed)
    assert q.shape == (1, 16, N, HD)

    key = scale_embed.tobytes()
    if key not in _CACHE:
        _CACHE.clear()
        _CACHE[key] = _build(scale_embed)
    nc = _CACHE[key]

    in_maps = _prep_inputs(q, k, v, pos_bias)
    res = run_bass_kernel_spmd(nc, in_maps, core_ids=list(range(8)), trace=TRACE)
    LAST_RESULTS[0] = res
    out = np.zeros((1, 16, N, HD), dtype=np.float32)
    for c in range(8):
        oT = res.results[c]["oT_out"]          # [128, N] bf16 (unnormalized)
        lv = res.results[c]["l_out"]           # [8, 1024] fp32
        # l[h, n]: n = 2048*pair + 512*hf + j -> l_out[2*hf+h, 512*pair+j]
        l = np.zeros((2, N), dtype=np.float32)
        for h in range(2):
            for pair in range(2):
                for hf in range(4):
                    n0 = 2048 * pair + 512 * hf
                    l[h, n0 : n0 + 512] = lv[2 * hf + h, 512 * pair : 512 * pair + 512]
        l = np.where(l > 0.0, l, 1.0)
        o = oT.astype(np.float32).reshape(2, HD, N).transpose(0, 2, 1)  # [2, N, HD]
        o = o / l[:, :, None]
        o[:, 0, :] = 0.0
        out[0, 2 * c : 2 * c + 2] = o
    return out



# revision 2
# speedup vs baseline: 1.1158x; 1.1158x over previous
"""Bass/Trainium2 kernel for nn_DSQGAttentionD41J16D (sparse offset attention).

Sharding: 16 heads over 8 cores -> 2 heads per core (SPMD). Host lays inputs
out transposed per core as [dh = h*64+d (128 partitions), 1024-pad + n] bf16,
so every offset-shift k[n-d_i] / v[n-d_i] is a free-dim slice on chip.

Window-pipelined schedule (window c ~= products of chunk c on DVE):
  DVE : [prods c0][prods c1][pvD c0][prods c2][pvD c1][prods c3][pvD c2][pvD c3]
  Pool: [pad memsets][5 early c3 products][pvP c0][pvP c1][pvP c2][pvP c3]
  PE  : [warm][sel c0][sel c1][acc c0 + psl c0][sel c2][acc c1 + psl c1] ...
  ACT : [q0/pb loads][exp c0][exp c1][outcopy c0 + out dma c0][exp c2] ...
  SP  : [k/q/v/blob loads][bcast c0 (pool-first)][bcast c1] ...

Scores: pss[(i,h), n] = sel-matmul partition-reduce of DVE products
q*k[n-d_i] plus an se-matmul, -1e30 validity mask for chunk 0; p = exp(
0.125*pss + pos_bias) on ACT (bias port). Denominator l via ones-matmuls
(PE) shipped to host; host divides and zeroes the n=0 row.
PV: p rows broadcast across the 64 d-partitions by repeat-read DMAs (SP
queue), tmp_i = p_bc * vT[n-d_i] split DVE/Pool, accumulated on PE via
identity matmuls into PSUM fp32.
"""

import os
import sys

sys.path.insert(0, "/opt/trn_rl_repo")

import numpy as np
import ml_dtypes

ALL_OFFSETS = [1, 3, 4, 13, 15, 21, 23, 28, 48, 64, 96, 192, 384, 512, 768, 1024]
N = 4096
HD = 64
P = 128
PAD = 1024
NT = PAD + N
NOFF = 16
C = 1024          # chunk width
NCH = N // C      # 4 chunks
NEG = -1.0e30

BF16 = ml_dtypes.bfloat16

# PV multiply jobs routed to the GPSIMD (Pool) engine: set of (chunk, offset).
POOL_PV = {(c, i) for c in range(NCH) for i in (1, 4, 7, 10, 13)}
# Score products computed early on Pool while it waits for the first
# broadcasts: (chunk, offset) pairs, all from the last chunk (inputs for
# chunk 3 need the full k/q load, ready by ~6us).
POOL_EARLY_PRODS = [(3, i) for i in (0, 2, 5, 8, 11)]

_CACHE = {}
TRACE = os.environ.get("BASS_KERNEL_TRACE", "0") == "1"
LAST_RESULTS = [None]

# const blob column layout
SEL0 = 0            # sel: 16 offsets x 32 cols
SET0 = 512          # seT: 32 cols
ID0 = 544           # ident: 128 cols
MKT0 = 672          # maskT: 32 cols (rows 0:16)
MK0 = 704           # mask01: 1024 cols (rows 0:16)
ONL0 = 1728         # onesl: 4 x 8 cols (rows 0:32)
BLOBW = 1760


def _build_blob(scale_embed_np):
    blob = np.zeros((P, BLOBW), dtype=BF16)
    for h in range(2):
        for d in range(HD):
            p = 64 * h + d
            for i in range(NOFF):
                blob[p, SEL0 + 32 * i + 2 * i + h] = 1.0
                blob[p, SET0 + 2 * i + h] = BF16(scale_embed_np[i, d])
    blob[:, ID0 : ID0 + P] = np.eye(P, dtype=BF16)
    for j in range(NOFF):
        blob[j, MKT0 + 2 * j] = NEG
        blob[j, MKT0 + 2 * j + 1] = NEG
        blob[j, MK0 : MK0 + ALL_OFFSETS[j]] = 1.0
    for hf in range(4):
        for i in range(NOFF):
            for h in range(2):
                blob[2 * i + h, ONL0 + 8 * hf + 2 * hf + h] = 1.0
    return blob


def _build(scale_embed_np):
    import concourse.bass as bass
    import concourse.mybir as mybir
    import concourse.tile as tile
    from concourse import bacc

    fp32 = mybir.dt.float32
    bf16 = mybir.dt.bfloat16
    MULT = mybir.AluOpType.mult
    EXP = mybir.ActivationFunctionType.Exp
    COPY = mybir.ActivationFunctionType.Copy

    nc = bacc.Bacc()

    qT_in = nc.dram_tensor("qT_in", [P, NT], bf16, kind="ExternalInput")
    kT_in = nc.dram_tensor("kT_in", [P, NT], bf16, kind="ExternalInput")
    vT_in = nc.dram_tensor("vT_in", [P, NT], bf16, kind="ExternalInput")
    pb_in = nc.dram_tensor("pb_in", [2 * NOFF, 1], bf16, kind="ExternalInput")
    oT_out = nc.dram_tensor("oT_out", [P, N], bf16, kind="ExternalOutput")
    l_out = nc.dram_tensor("l_out", [8, 1024], fp32, kind="ExternalOutput")

    blob_c = nc.inline_tensor(_build_blob(scale_embed_np), name="blob_c")

    with tile.TileContext(nc) as tc:
        consts = tc.alloc_tile_pool(name="consts", bufs=1)
        big = tc.alloc_tile_pool(name="big", bufs=1)
        ps_s = tc.alloc_tile_pool(name="ps_s", bufs=2, space="PSUM")
        ps_l = tc.alloc_tile_pool(name="ps_l", bufs=2, space="PSUM")
        ps_a = tc.alloc_tile_pool(name="ps_a", bufs=4, space="PSUM")
        work = tc.alloc_tile_pool(name="work", bufs=12)
        tmps = tc.alloc_tile_pool(name="tmps", bufs=10)
        bcast = tc.alloc_tile_pool(name="bcast", bufs=30)

        qT = big.tile([P, NT], bf16)
        kT = big.tile([P, NT], bf16)
        vT = big.tile([P, NT], bf16)
        p_sb = big.tile([2 * NOFF, N], bf16)
        out_sb = big.tile([P, N], bf16)
        l_sb = big.tile([8, 1024], fp32)
        blob = consts.tile([P, BLOBW], bf16)
        pb_sb = consts.tile([2 * NOFF, 1], bf16)

        # ---- loads: k chunk0 + q chunk0 first (first DVE products), then
        # full k/q (early Pool products need them), then v (PV phase).
        nc.sync.dma_start(out=kT[:, PAD : PAD + C], in_=kT_in[:, PAD : PAD + C])
        nc.scalar.dma_start(out=qT[:, PAD : PAD + C], in_=qT_in[:, PAD : PAD + C])
        nc.gpsimd.memset(kT[:, 0:PAD], 0.0)
        nc.gpsimd.memset(vT[:, 0:PAD], 0.0)
        nc.sync.dma_start(out=kT[:, PAD + C : NT], in_=kT_in[:, PAD + C : NT])
        nc.sync.dma_start(out=qT[:, PAD + C : NT], in_=qT_in[:, PAD + C : NT])
        nc.sync.dma_start(out=blob, in_=blob_c[:, :])
        nc.scalar.dma_start(out=pb_sb, in_=pb_in[:, :])
        nc.sync.dma_start(out=vT[:, PAD : NT], in_=vT_in[:, PAD : NT])

        # ---- blob slices ----
        def sel_w(i):
            return blob[:, SEL0 + 32 * i : SEL0 + 32 * i + 32]

        seT_w = blob[:, SET0 : SET0 + 32]
        ident_w = blob[:, ID0 : ID0 + P]
        maskT_w = blob[0:NOFF, MKT0 : MKT0 + 32]

        def mask01_x(g):
            return blob[0:NOFF, MK0 + 512 * g : MK0 + 512 * g + 512]

        def onesl_w(hf):
            return blob[0 : 2 * NOFF, ONL0 + 8 * hf : ONL0 + 8 * hf + 8]

        # single warm-up matmul: consumes the blob DMA semaphore wait early
        warm = ps_s.tile([P, 2], fp32, tag="pss", name="warm")
        nc.tensor.matmul(warm[0:32, 0:2], sel_w(0), ident_w[:, 0:2],
                         start=True, stop=True)

        # validity skip rule: offset i fully invalid in 512-chunk g iff
        # delta_i >= 512*(g+1)
        def sel_skip(g, i):
            return ALL_OFFSETS[i] >= 512 * (g + 1)

        def vstart(c, i):
            """512-aligned start of the valid region of (chunk c, offset i);
            >= C means fully skippable."""
            v = max(0, ALL_OFFSETS[i] - C * c)
            return (v // 512) * 512

        prod_tiles = {}

        def emit_product(c, i, eng):
            b = C * c
            d = ALL_OFFSETS[i]
            pr = work.tile([P, C], bf16, tag="prod", name=f"prod_{c}_{i}")
            eng.tensor_tensor(
                out=pr,
                in0=qT[:, PAD + b : PAD + b + C],
                in1=kT[:, PAD + b - d : PAD + b - d + C],
                op=MULT,
            )
            prod_tiles[(c, i)] = pr

        # ---- early Pool products (chunk 3) while Pool waits for bcasts ----
        for (c, i) in POOL_EARLY_PRODS:
            emit_product(c, i, nc.gpsimd)

        psl8 = [None, None]

        def emit_psl(c):
            """Denominator row-sum matmuls for chunk c's two 512-chunks."""
            pair = c // 2
            for gl in range(2):
                g = 2 * c + gl
                s0 = 512 * g
                hf = g - 4 * pair
                if hf == 0:
                    psl8[pair] = ps_l.tile(
                        [8, 512], fp32, tag="psl", name=f"psl_{pair}"
                    )
                nc.tensor.matmul(
                    psl8[pair], onesl_w(hf), p_sb[:, s0 : s0 + 512],
                    start=(hf == 0), stop=(hf == 3), skip_group_check=True,
                )

        def emit_scores(c):
            b = C * c
            g0, g1 = 2 * c, 2 * c + 1
            pss = {}
            for gl, g in ((0, g0), (1, g1)):
                pss[gl] = ps_s.tile(
                    [2 * NOFF, 512], fp32, tag="pss", name=f"pss_{g}"
                )
                nc.tensor.matmul(
                    pss[gl], seT_w,
                    qT[:, PAD + 512 * g : PAD + 512 * g + 512],
                    start=True, stop=False, skip_group_check=True,
                )
            valid0 = [i for i in range(NOFF) if not sel_skip(g0, i)]
            valid1 = [i for i in range(NOFF) if not sel_skip(g1, i)]
            masked = g0 < 2
            for i in range(NOFF):
                in0, in1 = i in valid0, i in valid1
                if not (in0 or in1):
                    continue
                if (c, i) not in prod_tiles:
                    emit_product(c, i, nc.vector)
                pr = prod_tiles[(c, i)]
                if in0:
                    nc.tensor.matmul(
                        pss[0], sel_w(i), pr[:, 0:512],
                        start=False,
                        stop=(not masked) and i == valid0[-1],
                        skip_group_check=True,
                    )
                if in1:
                    nc.tensor.matmul(
                        pss[1], sel_w(i), pr[:, 512:1024],
                        start=False,
                        stop=(not masked) and i == valid1[-1],
                        skip_group_check=True,
                    )
            if masked:
                for gl, g in ((0, g0), (1, g1)):
                    nc.tensor.matmul(
                        pss[gl], maskT_w, mask01_x(g),
                        start=False, stop=True, skip_group_check=True,
                    )
            for gl, g in ((0, g0), (1, g1)):
                nc.scalar.activation(
                    out=p_sb[:, 512 * g : 512 * g + 512], in_=pss[gl],
                    func=EXP, scale=0.125, bias=pb_sb[:, 0:1],
                )

        def pv_order(c):
            """PV offsets of chunk c: pool-share first, then DVE-share."""
            alive = [i for i in range(NOFF) if vstart(c, i) < C]
            return (
                [i for i in alive if (c, i) in POOL_PV]
                + [i for i in alive if (c, i) not in POOL_PV]
            )

        pbc_tiles = {}

        def emit_pbc(c):
            b = C * c
            for i in pv_order(c):
                v5 = vstart(c, i)
                w = C - v5
                rows = p_sb[2 * i : 2 * i + 2, b + v5 : b + v5 + w]
                dst = bcast.tile([P, w], bf16, tag="pbc", name=f"pbc_{c}_{i}")
                rep = bass.AP(
                    tensor=rows.tensor,
                    offset=rows.offset,
                    ap=[list(rows.ap[0]), [0, HD], [1, w]],
                )
                nc.sync.dma_start(out=dst, in_=rep)
                pbc_tiles[(c, i)] = dst

        acc = {}

        def emit_pv(c):
            """PV multiplies: Pool-share first (ready earliest — its bcasts
            were issued first), then DVE-share; identity-matmul accumulation
            follows each multiply on PE."""
            b = C * c
            order = pv_order(c)
            valid_g = [
                [i for i in order if vstart(c, i) <= 512 * gl] for gl in range(2)
            ]
            for gl in range(2):
                acc[2 * c + gl] = ps_a.tile(
                    [P, 512], fp32, tag="acc", name=f"acc_{2 * c + gl}"
                )
            started = [False, False]
            remaining = [len(valid_g[0]), len(valid_g[1])]
            for i in order:
                v5 = vstart(c, i)
                d = ALL_OFFSETS[i]
                w = C - v5
                tmp = tmps.tile([P, w], bf16, tag="tmp", name=f"tmp_{c}_{i}")
                eng = nc.gpsimd if (c, i) in POOL_PV else nc.vector
                eng.tensor_tensor(
                    out=tmp,
                    in0=pbc_tiles[(c, i)],
                    in1=vT[:, PAD + b + v5 - d : PAD + b + v5 - d + w],
                    op=MULT,
                )
                for gl in range(2):
                    if i not in valid_g[gl]:
                        continue
                    s0l = 512 * gl
                    remaining[gl] -= 1
                    nc.tensor.matmul(
                        acc[2 * c + gl], ident_w,
                        tmp[:, s0l - v5 : s0l - v5 + 512],
                        start=(not started[gl]),
                        stop=(remaining[gl] == 0),
                        skip_group_check=True,
                    )
                    started[gl] = True

        def emit_out(c):
            for gl in range(2):
                g = 2 * c + gl
                s0 = 512 * g
                nc.scalar.activation(
                    out=out_sb[:, s0 : s0 + 512], in_=acc[g], func=COPY
                )
            nc.scalar.dma_start(
                out=oT_out[:, C * c : C * c + C], in_=out_sb[:, C * c : C * c + C]
            )

        # ---- window-pipelined emission ----
        for c in range(NCH):
            emit_scores(c)
            emit_pbc(c)
            if c > 0:
                emit_pv(c - 1)
                emit_psl(c - 1)
                emit_out(c - 1)
        emit_pv(NCH - 1)
        emit_psl(NCH - 1)
        emit_out(NCH - 1)
        for pair in range(2):
            nc.scalar.activation(
                out=l_sb[:, 512 * pair : 512 * pair + 512],
                in_=psl8[pair], func=COPY,
            )
        nc.scalar.dma_start(out=l_out[:, :], in_=l_sb)

        bcast.release()
        tmps.release()
        work.release()
        ps_a.release()
        ps_l.release()
        ps_s.release()
        big.release()
        consts.release()

    nc.compile()
    return nc


def _prep_inputs(q, k, v, pos_bias):
    """Host-side sharding + layout prep: per core, heads (2c, 2c+1) packed as
    128 partitions (h*64+d), transposed to [dh, pad+n] bf16."""
    def to_T(x):
        xt = np.ascontiguousarray(x[0].transpose(0, 2, 1)).astype(BF16)
        xt = xt.reshape(8, P, N)
        return np.concatenate([np.zeros((8, P, PAD), dtype=BF16), xt], axis=2)

    qT = to_T(q)
    kT = to_T(k)
    vT = to_T(v)

    in_maps = []
    for c in range(8):
        pb = np.zeros((2 * NOFF, 1), dtype=np.float32)
        for i in range(NOFF):
            for hh in range(2):
                pb[2 * i + hh, 0] = pos_bias[i, 2 * c + hh]
        in_maps.append(
            {
                "qT_in": qT[c],
                "kT_in": kT[c],
                "vT_in": vT[c],
                "pb_in": pb.astype(BF16),
            }
        )
    return in_maps


def kernel(q, k, v, pos_bias, scale_embed):
    from concourse.bass_utils import run_bass_kernel_spmd

    q = np.asarray(q)
    k = np.asarray(k)
    v = np.asarray(v)
    pos_bias = np.asarray(pos_bias)
    scale_embed = np.asarray(scale_embed)
    assert q.shape == (1, 16, N, HD)

    key = scale_embed.tobytes()
    if key not in _CACHE:
        _CACHE.clear()
        _CACHE[key] = _build(scale_embed)
    nc = _CACHE[key]

    in_maps = _prep_inputs(q, k, v, pos_bias)
    res = run_bass_kernel_spmd(nc, in_maps, core_ids=list(range(8)), trace=TRACE)
    LAST_RESULTS[0] = res
    out = np.zeros((1, 16, N, HD), dtype=np.float32)
    for c in range(8):
        oT = res.results[c]["oT_out"]          # [128, N] bf16 (unnormalized)
        lv = res.results[c]["l_out"]           # [8, 1024] fp32
        # l[h, n]: n = 2048*pair + 512*hf + j -> l_out[2*hf+h, 512*pair+j]
        l = np.zeros((2, N), dtype=np.float32)
        for h in range(2):
            for pair in range(2):
                for hf in range(4):
                    n0 = 2048 * pair + 512 * hf
                    l[h, n0 : n0 + 512] = lv[2 * hf + h, 512 * pair : 512 * pair + 512]
        l = np.where(l > 0.0, l, 1.0)
        o = oT.astype(np.float32).reshape(2, HD, N).transpose(0, 2, 1)  # [2, N, HD]
        o = o / l[:, :, None]
        o[:, 0, :] = 0.0
        out[0, 2 * c : 2 * c + 2] = o
    return out


# revision 7
# speedup vs baseline: 1.1965x; 1.0723x over previous
"""Bass/Trainium2 kernel for nn_DSQGAttentionD41J16D (sparse offset attention).

Sharding: 16 heads over 8 cores -> 2 heads per core (SPMD). Host lays inputs
out transposed per core as [dh = h*64+d (128 partitions), 1024-pad + n] bf16,
so every offset-shift k[n-d_i] / v[n-d_i] is a free-dim slice on chip.

Window-pipelined schedule (window c ~= products of chunk c on DVE):
  DVE : [prods c0][prods c1][pvD c0][prods c2][pvD c1][prods c3][pvD c2][pvD c3]
  Pool: [pad memsets][5 early c3 products][pvP c0][pvP c1][pvP c2][pvP c3]
  PE  : [warm][sel c0][sel c1][acc c0 + psl c0][sel c2][acc c1 + psl c1] ...
  ACT : [q0/pb loads][exp c0][exp c1][outcopy c0 + out dma c0][exp c2] ...
  SP  : [k/q/v/blob loads][bcast c0 (pool-first)][bcast c1] ...

Scores: pss[(i,h), n] = sel-matmul partition-reduce of DVE products
q*k[n-d_i] plus an se-matmul, -1e30 validity mask for chunk 0; p = exp(
0.125*pss + pos_bias) on ACT (bias port). Denominator l via ones-matmuls
(PE) shipped to host; host divides and zeroes the n=0 row.
PV: p rows broadcast across the 64 d-partitions by repeat-read DMAs (SP
queue), tmp_i = p_bc * vT[n-d_i] split DVE/Pool, accumulated on PE via
identity matmuls into PSUM fp32.
"""

import os
import sys

sys.path.insert(0, "/opt/trn_rl_repo")

import numpy as np
import ml_dtypes

ALL_OFFSETS = [1, 3, 4, 13, 15, 21, 23, 28, 48, 64, 96, 192, 384, 512, 768, 1024]
N = 4096
HD = 64
P = 128
PAD = 1024
NT = PAD + N
NOFF = 16
C = 1024          # chunk width
NCH = N // C      # 4 chunks
NEG = -1.0e30

BF16 = ml_dtypes.bfloat16

# PV multiply jobs routed to the GPSIMD (Pool) engine: (chunk, offset).
POOL_PV = (
    {(0, i) for i in (1, 4, 7, 10)}
    | {(1, i) for i in (1, 4, 7, 10)}
    | {(2, i) for i in (1, 4, 7, 10, 13)}
    | {(3, i) for i in (1, 4, 7, 10, 13)}
)
# Score products computed on Pool: chunk-1 products run while Pool waits for
# the first broadcasts (chunk-1 k/q pieces load right after chunk 0); later
# chunks' products fill Pool's idle time inside each window.
POOL_PRODS = {1: (0, 2, 5, 8, 11), 2: (0, 5), 3: (0, 5)}

_CACHE = {}
TRACE = os.environ.get("BASS_KERNEL_TRACE", "0") == "1"
LAST_RESULTS = [None]

# const blob column layout
SEL0 = 0            # sel: 16 offsets x 32 cols
SET0 = 512          # seT: 32 cols
ID0 = 544           # ident: 128 cols
MKT0 = 672          # maskT: 32 cols (rows 0:16)
MK0 = 704           # mask01: 1024 cols (rows 0:16)
ONL0 = 1728         # onesl: 4 x 8 cols (rows 0:32)
BLOBW = 1760


def _build_blob(scale_embed_np):
    blob = np.zeros((P, BLOBW), dtype=BF16)
    for h in range(2):
        for d in range(HD):
            p = 64 * h + d
            for i in range(NOFF):
                blob[p, SEL0 + 32 * i + 2 * i + h] = 1.0
                blob[p, SET0 + 2 * i + h] = BF16(scale_embed_np[i, d])
    blob[:, ID0 : ID0 + P] = np.eye(P, dtype=BF16)
    for j in range(NOFF):
        blob[j, MKT0 + 2 * j] = NEG
        blob[j, MKT0 + 2 * j + 1] = NEG
        blob[j, MK0 : MK0 + ALL_OFFSETS[j]] = 1.0
    for hf in range(4):
        for i in range(NOFF):
            for h in range(2):
                blob[2 * i + h, ONL0 + 8 * hf + 2 * hf + h] = 1.0
    return blob


def _build(scale_embed_np):
    import concourse.bass as bass
    import concourse.mybir as mybir
    import concourse.tile as tile
    from concourse import bacc

    fp32 = mybir.dt.float32
    bf16 = mybir.dt.bfloat16
    MULT = mybir.AluOpType.mult
    EXP = mybir.ActivationFunctionType.Exp
    COPY = mybir.ActivationFunctionType.Copy

    nc = bacc.Bacc()

    qT_in = nc.dram_tensor("qT_in", [P, NT], bf16, kind="ExternalInput")
    kT_in = nc.dram_tensor("kT_in", [P, NT], bf16, kind="ExternalInput")
    vT_in = nc.dram_tensor("vT_in", [P, NT], bf16, kind="ExternalInput")
    pb_in = nc.dram_tensor("pb_in", [2 * NOFF, 1], bf16, kind="ExternalInput")
    oT_out = nc.dram_tensor("oT_out", [P, N], bf16, kind="ExternalOutput")
    l_out = nc.dram_tensor("l_out", [8, 1024], fp32, kind="ExternalOutput")

    blob_c = nc.inline_tensor(_build_blob(scale_embed_np), name="blob_c")

    with tile.TileContext(nc) as tc:
        consts = tc.alloc_tile_pool(name="consts", bufs=1)
        big = tc.alloc_tile_pool(name="big", bufs=1)
        ps_s = tc.alloc_tile_pool(name="ps_s", bufs=2, space="PSUM")
        ps_l = tc.alloc_tile_pool(name="ps_l", bufs=2, space="PSUM")
        ps_a = tc.alloc_tile_pool(name="ps_a", bufs=4, space="PSUM")
        work = tc.alloc_tile_pool(name="work", bufs=22)
        tmps = tc.alloc_tile_pool(name="tmps", bufs=10)
        bcast = tc.alloc_tile_pool(name="bcast", bufs=30)

        qT = big.tile([P, NT], bf16)
        kT = big.tile([P, NT], bf16)
        vT = big.tile([P, NT], bf16)
        p_sb = big.tile([2 * NOFF, N], bf16)
        out_sb = big.tile([P, N], bf16)
        l_sb = big.tile([8, 1024], fp32)
        blob = consts.tile([P, BLOBW], bf16)
        pb_sb = consts.tile([2 * NOFF, 1], bf16)

        # ---- loads: k0+q0 first (first DVE products), blob (PE warm/sel),
        # then k1/q1 (early Pool products), the rest, then v (PV phase).
        nc.sync.dma_start(out=kT[:, PAD : PAD + C], in_=kT_in[:, PAD : PAD + C])
        nc.scalar.dma_start(out=qT[:, PAD : PAD + C], in_=qT_in[:, PAD : PAD + C])
        nc.gpsimd.memset(kT[:, 0:PAD], 0.0)
        nc.gpsimd.memset(vT[:, 0:PAD], 0.0)
        nc.sync.dma_start(out=blob, in_=blob_c[:, :])
        nc.scalar.dma_start(out=pb_sb, in_=pb_in[:, :])
        nc.sync.dma_start(out=kT[:, PAD + C : PAD + 2 * C], in_=kT_in[:, PAD + C : PAD + 2 * C])
        nc.sync.dma_start(out=qT[:, PAD + C : PAD + 2 * C], in_=qT_in[:, PAD + C : PAD + 2 * C])
        nc.sync.dma_start(out=kT[:, PAD + 2 * C : NT], in_=kT_in[:, PAD + 2 * C : NT])
        nc.sync.dma_start(out=qT[:, PAD + 2 * C : NT], in_=qT_in[:, PAD + 2 * C : NT])
        nc.sync.dma_start(out=vT[:, PAD : NT], in_=vT_in[:, PAD : NT])

        # ---- blob slices ----
        def sel_w(i):
            return blob[:, SEL0 + 32 * i : SEL0 + 32 * i + 32]

        seT_w = blob[:, SET0 : SET0 + 32]
        ident_w = blob[:, ID0 : ID0 + P]
        maskT_w = blob[0:NOFF, MKT0 : MKT0 + 32]

        def mask01_x(g):
            return blob[0:NOFF, MK0 + 512 * g : MK0 + 512 * g + 512]

        def onesl_w(hf):
            return blob[0 : 2 * NOFF, ONL0 + 8 * hf : ONL0 + 8 * hf + 8]

        # single warm-up matmul: consumes the blob DMA semaphore wait early
        warm = ps_s.tile([P, 2], fp32, tag="pss", name="warm")
        nc.tensor.matmul(warm[0:32, 0:2], sel_w(0), ident_w[:, 0:2],
                         start=True, stop=True)

        # validity skip rule: offset i fully invalid in 512-chunk g iff
        # delta_i >= 512*(g+1)
        def sel_skip(g, i):
            return ALL_OFFSETS[i] >= 512 * (g + 1)

        def vstart(c, i):
            """512-aligned start of the valid region of (chunk c, offset i);
            >= C means fully skippable."""
            v = max(0, ALL_OFFSETS[i] - C * c)
            return (v // 512) * 512

        prod_tiles = {}

        def emit_product(c, i, eng):
            b = C * c
            d = ALL_OFFSETS[i]
            pr = work.tile([P, C], bf16, tag="prod", name=f"prod_{c}_{i}")
            eng.tensor_tensor(
                out=pr,
                in0=qT[:, PAD + b : PAD + b + C],
                in1=kT[:, PAD + b - d : PAD + b - d + C],
                op=MULT,
            )
            prod_tiles[(c, i)] = pr

        # ---- early Pool products (chunk 1) while Pool waits for bcasts ----
        for i in POOL_PRODS[1]:
            emit_product(1, i, nc.gpsimd)

        psl8 = [None, None]

        def emit_psl(c):
            """Denominator row-sum matmuls for chunk c's two 512-chunks."""
            pair = c // 2
            for gl in range(2):
                g = 2 * c + gl
                s0 = 512 * g
                hf = g - 4 * pair
                if hf == 0:
                    psl8[pair] = ps_l.tile(
                        [8, 512], fp32, tag="psl", name=f"psl_{pair}"
                    )
                nc.tensor.matmul(
                    psl8[pair], onesl_w(hf), p_sb[:, s0 : s0 + 512],
                    start=(hf == 0), stop=(hf == 3), skip_group_check=True,
                )

        def emit_scores(c):
            b = C * c
            g0, g1 = 2 * c, 2 * c + 1
            pss = {}
            for gl, g in ((0, g0), (1, g1)):
                pss[gl] = ps_s.tile(
                    [2 * NOFF, 512], fp32, tag="pss", name=f"pss_{g}"
                )
                nc.tensor.matmul(
                    pss[gl], seT_w,
                    qT[:, PAD + 512 * g : PAD + 512 * g + 512],
                    start=True, stop=False, skip_group_check=True,
                )
            valid0 = [i for i in range(NOFF) if not sel_skip(g0, i)]
            valid1 = [i for i in range(NOFF) if not sel_skip(g1, i)]
            masked = g0 < 2
            for i in range(NOFF):
                in0, in1 = i in valid0, i in valid1
                if not (in0 or in1):
                    continue
                if (c, i) not in prod_tiles:
                    emit_product(c, i, nc.vector)
                pr = prod_tiles[(c, i)]
                if in0:
                    nc.tensor.matmul(
                        pss[0], sel_w(i), pr[:, 0:512],
                        start=False,
                        stop=(not masked) and i == valid0[-1],
                        skip_group_check=True,
                    )
                if in1:
                    nc.tensor.matmul(
                        pss[1], sel_w(i), pr[:, 512:1024],
                        start=False,
                        stop=(not masked) and i == valid1[-1],
                        skip_group_check=True,
                    )
            if masked:
                for gl, g in ((0, g0), (1, g1)):
                    nc.tensor.matmul(
                        pss[gl], maskT_w, mask01_x(g),
                        start=False, stop=True, skip_group_check=True,
                    )
            for gl, g in ((0, g0), (1, g1)):
                nc.scalar.activation(
                    out=p_sb[:, 512 * g : 512 * g + 512], in_=pss[gl],
                    func=EXP, scale=0.125, bias=pb_sb[:, 0:1],
                )

        def pv_order(c):
            """PV offsets of chunk c: pool-share first, then DVE-share."""
            alive = [i for i in range(NOFF) if vstart(c, i) < C]
            return (
                [i for i in alive if (c, i) in POOL_PV]
                + [i for i in alive if (c, i) not in POOL_PV]
            )

        pbc_tiles = {}

        def emit_pbc(c):
            b = C * c
            for i in pv_order(c):
                v5 = vstart(c, i)
                w = C - v5
                rows = p_sb[2 * i : 2 * i + 2, b + v5 : b + v5 + w]
                dst = bcast.tile([P, w], bf16, tag="pbc", name=f"pbc_{c}_{i}")
                rep = bass.AP(
                    tensor=rows.tensor,
                    offset=rows.offset,
                    ap=[list(rows.ap[0]), [0, HD], [1, w]],
                )
                nc.sync.dma_start(out=dst, in_=rep)
                pbc_tiles[(c, i)] = dst

        acc = {}

        def emit_pv(c):
            """PV multiplies: Pool-share first (ready earliest — its bcasts
            were issued first), then DVE-share; identity-matmul accumulation
            follows each multiply on PE."""
            b = C * c
            order = pv_order(c)
            valid_g = [
                [i for i in order if vstart(c, i) <= 512 * gl] for gl in range(2)
            ]
            for gl in range(2):
                acc[2 * c + gl] = ps_a.tile(
                    [P, 512], fp32, tag="acc", name=f"acc_{2 * c + gl}"
                )
            started = [False, False]
            remaining = [len(valid_g[0]), len(valid_g[1])]
            for i in order:
                v5 = vstart(c, i)
                d = ALL_OFFSETS[i]
                w = C - v5
                tmp = tmps.tile([P, w], bf16, tag="tmp", name=f"tmp_{c}_{i}")
                eng = nc.gpsimd if (c, i) in POOL_PV else nc.vector
                eng.tensor_tensor(
                    out=tmp,
                    in0=pbc_tiles[(c, i)],
                    in1=vT[:, PAD + b + v5 - d : PAD + b + v5 - d + w],
                    op=MULT,
                )
                for gl in range(2):
                    if i not in valid_g[gl]:
                        continue
                    s0l = 512 * gl
                    remaining[gl] -= 1
                    nc.tensor.matmul(
                        acc[2 * c + gl], ident_w,
                        tmp[:, s0l - v5 : s0l - v5 + 512],
                        start=(not started[gl]),
                        stop=(remaining[gl] == 0),
                        skip_group_check=True,
                    )
                    started[gl] = True

        def emit_out(c):
            # per-half copy + DMA so the first half's output ships while the
            # second half is still copying (shortens the tail)
            for gl in range(2):
                g = 2 * c + gl
                s0 = 512 * g
                nc.scalar.activation(
                    out=out_sb[:, s0 : s0 + 512], in_=acc[g], func=COPY
                )
                nc.scalar.dma_start(
                    out=oT_out[:, s0 : s0 + 512], in_=out_sb[:, s0 : s0 + 512]
                )

        def emit_lcopy(pair):
            nc.scalar.activation(
                out=l_sb[:, 512 * pair : 512 * pair + 512],
                in_=psl8[pair], func=COPY,
            )

        # ---- window-pipelined emission ----
        for c in range(NCH):
            emit_scores(c)
            emit_pbc(c)
            if c > 0:
                emit_psl(c - 1)          # PE filler; also releases l pair 0 early
                if c - 1 == 1:
                    emit_lcopy(0)        # psl pair 0 complete after psl(c1)
                emit_pv(c - 1)
                for i in POOL_PRODS.get(c + 1, ()):
                    emit_product(c + 1, i, nc.gpsimd)
                emit_out(c - 1)
        emit_psl(NCH - 1)
        emit_lcopy(1)
        emit_pv(NCH - 1)
        emit_out(NCH - 1)
        nc.scalar.dma_start(out=l_out[:, :], in_=l_sb)

        bcast.release()
        tmps.release()
        work.release()
        ps_a.release()
        ps_l.release()
        ps_s.release()
        big.release()
        consts.release()

    nc.compile()
    return nc


def _prep_inputs(q, k, v, pos_bias):
    """Host-side sharding + layout prep: per core, heads (2c, 2c+1) packed as
    128 partitions (h*64+d), transposed to [dh, pad+n] bf16."""
    def to_T(x):
        xt = np.ascontiguousarray(x[0].transpose(0, 2, 1)).astype(BF16)
        xt = xt.reshape(8, P, N)
        return np.concatenate([np.zeros((8, P, PAD), dtype=BF16), xt], axis=2)

    qT = to_T(q)
    kT = to_T(k)
    vT = to_T(v)

    in_maps = []
    for c in range(8):
        pb = np.zeros((2 * NOFF, 1), dtype=np.float32)
        for i in range(NOFF):
            for hh in range(2):
                pb[2 * i + hh, 0] = pos_bias[i, 2 * c + hh]
        in_maps.append(
            {
                "qT_in": qT[c],
                "kT_in": kT[c],
                "vT_in": vT[c],
                "pb_in": pb.astype(BF16),
            }
        )
    return in_maps


def kernel(q, k, v, pos_bias, scale_embed):
    from concourse.bass_utils import run_bass_kernel_spmd

    q = np.asarray(q)
    k = np.asarray(k)
    v = np.asarray(v)
    pos_bias = np.asarray(pos_bias)
    scale_embed = np.asarray(scale_embed)
    assert q.shape == (1, 16, N, HD)

    key = scale_embed.tobytes()
    if key not in _CACHE:
        _CACHE.clear()
        _CACHE[key] = _build(scale_embed)
    nc = _CACHE[key]

    in_maps = _prep_inputs(q, k, v, pos_bias)
    res = run_bass_kernel_spmd(nc, in_maps, core_ids=list(range(8)), trace=TRACE)
    LAST_RESULTS[0] = res
    out = np.zeros((1, 16, N, HD), dtype=np.float32)
    for c in range(8):
        oT = res.results[c]["oT_out"]          # [128, N] bf16 (unnormalized)
        lv = res.results[c]["l_out"]           # [8, 1024] fp32
        # l[h, n]: n = 2048*pair + 512*hf + j -> l_out[2*hf+h, 512*pair+j]
        l = np.zeros((2, N), dtype=np.float32)
        for h in range(2):
            for pair in range(2):
                for hf in range(4):
                    n0 = 2048 * pair + 512 * hf
                    l[h, n0 : n0 + 512] = lv[2 * hf + h, 512 * pair : 512 * pair + 512]
        l = np.where(l > 0.0, l, 1.0)
        o = oT.astype(np.float32).reshape(2, HD, N).transpose(0, 2, 1)  # [2, N, HD]
        o = o / l[:, :, None]
        o[:, 0, :] = 0.0
        out[0, 2 * c : 2 * c + 2] = o
    return out


# revision 12
# speedup vs baseline: 1.2335x; 1.0310x over previous
"""Bass/Trainium2 kernel for nn_DSQGAttentionD41J16D (sparse offset attention).

Sharding: 16 heads over 8 cores -> 2 heads per core (SPMD). Host lays inputs
out transposed per core as [dh = h*64+d (128 partitions), 1024-pad + n] bf16,
so every offset-shift k[n-d_i] / v[n-d_i] is a free-dim slice on chip.

Window-pipelined schedule (window c ~= products of chunk c on DVE):
  DVE : [prods c0][prods c1][pvD c0][prods c2][pvD c1][prods c3][pvD c2][pvD c3]
  Pool: [pad memsets][5 early c3 products][pvP c0][pvP c1][pvP c2][pvP c3]
  PE  : [warm][sel c0][sel c1][acc c0 + psl c0][sel c2][acc c1 + psl c1] ...
  ACT : [q0/pb loads][exp c0][exp c1][outcopy c0 + out dma c0][exp c2] ...
  SP  : [k/q/v/blob loads][bcast c0 (pool-first)][bcast c1] ...

Scores: pss[(i,h), n] = sel-matmul partition-reduce of DVE products
q*k[n-d_i] plus an se-matmul, -1e30 validity mask for chunk 0; p = exp(
0.125*pss + pos_bias) on ACT (bias port). Denominator l via ones-matmuls
(PE) shipped to host; host divides and zeroes the n=0 row.
PV: p rows broadcast across the 64 d-partitions by repeat-read DMAs (SP
queue), tmp_i = p_bc * vT[n-d_i] split DVE/Pool, accumulated on PE via
identity matmuls into PSUM fp32.
"""

import os
import sys

sys.path.insert(0, "/opt/trn_rl_repo")

import numpy as np
import ml_dtypes

ALL_OFFSETS = [1, 3, 4, 13, 15, 21, 23, 28, 48, 64, 96, 192, 384, 512, 768, 1024]
N = 4096
HD = 64
P = 128
PAD = 1024
NT = PAD + N
NOFF = 16
C = 1024          # chunk width
NCH = N // C      # 4 chunks
NEG = -1.0e30

BF16 = ml_dtypes.bfloat16

# PV multiply jobs routed to the GPSIMD (Pool) engine: (chunk, offset).
POOL_PV = (
    {(0, i) for i in (1, 4, 7, 10)}
    | {(1, i) for i in (1, 4, 7, 10)}
    | {(2, i) for i in (1, 4, 7, 10)}
    | {(3, i) for i in (1, 4, 7, 10)}
)
# Score products computed on Pool up front (chunk-1 pieces of k/q load right
# after chunk 0, so Pool streams products while waiting for broadcasts).
POOL_PRODS = {1: (0, 2, 5, 8, 11, 14), 2: (0, 5), 3: (0, 5)}

_CACHE = {}
TRACE = os.environ.get("BASS_KERNEL_TRACE", "0") == "1"
LAST_RESULTS = [None]

# const blob column layout
SEL0 = 0            # sel: 16 offsets x 32 cols
SET0 = 512          # seT: 32 cols
ID0 = 544           # ident: 128 cols
MKT0 = 672          # maskT: 32 cols (rows 0:16)
MK0 = 704           # mask01: 1024 cols (rows 0:16)
ONL0 = 1728         # onesl: 4 x 8 cols (rows 0:32)
BLOBW = 1760


def _build_blob(scale_embed_np):
    blob = np.zeros((P, BLOBW), dtype=BF16)
    for h in range(2):
        for d in range(HD):
            p = 64 * h + d
            for i in range(NOFF):
                blob[p, SEL0 + 32 * i + 2 * i + h] = 1.0
                blob[p, SET0 + 2 * i + h] = BF16(scale_embed_np[i, d])
    blob[:, ID0 : ID0 + P] = np.eye(P, dtype=BF16)
    for j in range(NOFF):
        blob[j, MKT0 + 2 * j] = NEG
        blob[j, MKT0 + 2 * j + 1] = NEG
        blob[j, MK0 : MK0 + ALL_OFFSETS[j]] = 1.0
    for hf in range(4):
        for i in range(NOFF):
            for h in range(2):
                blob[2 * i + h, ONL0 + 8 * hf + 2 * hf + h] = 1.0
    return blob


def _build(scale_embed_np):
    import concourse.bass as bass
    import concourse.mybir as mybir
    import concourse.tile as tile
    from concourse import bacc

    fp32 = mybir.dt.float32
    bf16 = mybir.dt.bfloat16
    MULT = mybir.AluOpType.mult
    EXP = mybir.ActivationFunctionType.Exp
    COPY = mybir.ActivationFunctionType.Copy

    nc = bacc.Bacc()

    qT_in = nc.dram_tensor("qT_in", [P, NT], bf16, kind="ExternalInput")
    kT_in = nc.dram_tensor("kT_in", [P, NT], bf16, kind="ExternalInput")
    vT_in = nc.dram_tensor("vT_in", [P, NT], bf16, kind="ExternalInput")
    pb_in = nc.dram_tensor("pb_in", [2 * NOFF, 1], bf16, kind="ExternalInput")
    oT_out = nc.dram_tensor("oT_out", [P, N], bf16, kind="ExternalOutput")
    l_out = nc.dram_tensor("l_out", [8, 1024], fp32, kind="ExternalOutput")

    blob_c = nc.inline_tensor(_build_blob(scale_embed_np), name="blob_c")

    with tile.TileContext(nc) as tc:
        consts = tc.alloc_tile_pool(name="consts", bufs=1)
        big = tc.alloc_tile_pool(name="big", bufs=1)
        ps_s = tc.alloc_tile_pool(name="ps_s", bufs=2, space="PSUM")
        ps_l = tc.alloc_tile_pool(name="ps_l", bufs=2, space="PSUM")
        ps_a = tc.alloc_tile_pool(name="ps_a", bufs=4, space="PSUM")
        work = tc.alloc_tile_pool(name="work", bufs=22)
        tmps = tc.alloc_tile_pool(name="tmps", bufs=20)
        bcast = tc.alloc_tile_pool(name="bcast", bufs=30)

        qT = big.tile([P, NT], bf16)
        kT = big.tile([P, NT], bf16)
        vT = big.tile([P, NT], bf16)
        p_sb = big.tile([2 * NOFF, N], bf16)
        out_sb = big.tile([P, N], bf16)
        l_sb = big.tile([8, 1024], fp32)
        blob = consts.tile([P, BLOBW], bf16)
        pb_sb = consts.tile([2 * NOFF, 1], bf16)

        # ---- loads: q0 + k0 halves first (the first chunk-0 products d=512,
        # 768 need only q0 + pad + the first k0 half), then k1/q1 (early Pool
        # products), blob (PE warm/sel), the rest, then v (PV phase).
        nc.scalar.dma_start(out=qT[:, PAD : PAD + C], in_=qT_in[:, PAD : PAD + C])
        nc.sync.dma_start(out=kT[:, PAD : PAD + 512], in_=kT_in[:, PAD : PAD + 512])
        nc.gpsimd.memset(kT[:, 0:PAD], 0.0)
        nc.gpsimd.memset(vT[:, 0:PAD], 0.0)
        nc.sync.dma_start(out=kT[:, PAD + 512 : PAD + C], in_=kT_in[:, PAD + 512 : PAD + C])
        nc.sync.dma_start(out=kT[:, PAD + C : PAD + 2 * C], in_=kT_in[:, PAD + C : PAD + 2 * C])
        nc.sync.dma_start(out=qT[:, PAD + C : PAD + 2 * C], in_=qT_in[:, PAD + C : PAD + 2 * C])
        nc.sync.dma_start(out=blob, in_=blob_c[:, :])
        nc.scalar.dma_start(out=pb_sb, in_=pb_in[:, :])
        nc.sync.dma_start(out=kT[:, PAD + 2 * C : NT], in_=kT_in[:, PAD + 2 * C : NT])
        nc.sync.dma_start(out=qT[:, PAD + 2 * C : NT], in_=qT_in[:, PAD + 2 * C : NT])
        nc.sync.dma_start(out=vT[:, PAD : NT], in_=vT_in[:, PAD : NT])

        # ---- blob slices ----
        def sel_w(i):
            return blob[:, SEL0 + 32 * i : SEL0 + 32 * i + 32]

        seT_w = blob[:, SET0 : SET0 + 32]
        ident_w = blob[:, ID0 : ID0 + P]
        maskT_w = blob[0:NOFF, MKT0 : MKT0 + 32]

        def mask01_x(g):
            return blob[0:NOFF, MK0 + 512 * g : MK0 + 512 * g + 512]

        def onesl_w(hf):
            return blob[0 : 2 * NOFF, ONL0 + 8 * hf : ONL0 + 8 * hf + 8]

        # single warm-up matmul: consumes the blob DMA semaphore wait early
        warm = ps_s.tile([P, 2], fp32, tag="pss", name="warm")
        nc.tensor.matmul(warm[0:32, 0:2], sel_w(0), ident_w[:, 0:2],
                         start=True, stop=True)

        # validity skip rule: offset i fully invalid in 512-chunk g iff
        # delta_i >= 512*(g+1)
        def sel_skip(g, i):
            return ALL_OFFSETS[i] >= 512 * (g + 1)

        def vstart(c, i):
            """512-aligned start of the valid region of (chunk c, offset i);
            >= C means fully skippable."""
            v = max(0, ALL_OFFSETS[i] - C * c)
            return (v // 512) * 512

        prod_tiles = {}

        def emit_product(c, i, eng):
            b = C * c
            d = ALL_OFFSETS[i]
            pr = work.tile([P, C], bf16, tag="prod", name=f"prod_{c}_{i}")
            eng.tensor_tensor(
                out=pr,
                in0=qT[:, PAD + b : PAD + b + C],
                in1=kT[:, PAD + b - d : PAD + b - d + C],
                op=MULT,
            )
            prod_tiles[(c, i)] = pr

        # ---- early Pool products (chunk 1) while Pool waits for bcasts ----
        for i in POOL_PRODS[1]:
            emit_product(1, i, nc.gpsimd)

        psl8 = [None, None]

        def emit_psl(c):
            """Denominator row-sum matmuls for chunk c's two 512-chunks."""
            pair = c // 2
            for gl in range(2):
                g = 2 * c + gl
                s0 = 512 * g
                hf = g - 4 * pair
                if hf == 0:
                    psl8[pair] = ps_l.tile(
                        [8, 512], fp32, tag="psl", name=f"psl_{pair}"
                    )
                nc.tensor.matmul(
                    psl8[pair], onesl_w(hf), p_sb[:, s0 : s0 + 512],
                    start=(hf == 0), stop=(hf == 3), skip_group_check=True,
                )

        def emit_scores(c):
            b = C * c
            g0, g1 = 2 * c, 2 * c + 1
            pss = {}
            for gl, g in ((0, g0), (1, g1)):
                pss[gl] = ps_s.tile(
                    [2 * NOFF, 512], fp32, tag="pss", name=f"pss_{g}"
                )
                nc.tensor.matmul(
                    pss[gl], seT_w,
                    qT[:, PAD + 512 * g : PAD + 512 * g + 512],
                    start=True, stop=False, skip_group_check=True,
                )
            # chunk 0: products d=512/768 first — they need only the first
            # half of k0, so DVE starts ~1us earlier
            iorder = (
                [13, 14] + [i for i in range(NOFF) if i not in (13, 14)]
                if c == 0 else list(range(NOFF))
            )
            valid0 = [i for i in iorder if not sel_skip(g0, i)]
            valid1 = [i for i in iorder if not sel_skip(g1, i)]
            masked = g0 < 2
            for i in iorder:
                in0, in1 = i in valid0, i in valid1
                if not (in0 or in1):
                    continue
                if (c, i) not in prod_tiles:
                    emit_product(c, i, nc.vector)
                pr = prod_tiles[(c, i)]
                if in0:
                    nc.tensor.matmul(
                        pss[0], sel_w(i), pr[:, 0:512],
                        start=False,
                        stop=(not masked) and i == valid0[-1],
                        skip_group_check=True,
                    )
                if in1:
                    nc.tensor.matmul(
                        pss[1], sel_w(i), pr[:, 512:1024],
                        start=False,
                        stop=(not masked) and i == valid1[-1],
                        skip_group_check=True,
                    )
            if masked:
                for gl, g in ((0, g0), (1, g1)):
                    nc.tensor.matmul(
                        pss[gl], maskT_w, mask01_x(g),
                        start=False, stop=True, skip_group_check=True,
                    )
            for gl, g in ((0, g0), (1, g1)):
                nc.scalar.activation(
                    out=p_sb[:, 512 * g : 512 * g + 512], in_=pss[gl],
                    func=EXP, scale=0.125, bias=pb_sb[:, 0:1],
                )

        def pv_order(c):
            """PV offsets of chunk c: pool-share first, then DVE-share."""
            alive = [i for i in range(NOFF) if vstart(c, i) < C]
            return (
                [i for i in alive if (c, i) in POOL_PV]
                + [i for i in alive if (c, i) not in POOL_PV]
            )

        pbc_tiles = {}

        def emit_pbc(c):
            b = C * c
            for i in pv_order(c):
                v5 = vstart(c, i)
                w = C - v5
                rows = p_sb[2 * i : 2 * i + 2, b + v5 : b + v5 + w]
                dst = bcast.tile([P, w], bf16, tag="pbc", name=f"pbc_{c}_{i}")
                rep = bass.AP(
                    tensor=rows.tensor,
                    offset=rows.offset,
                    ap=[list(rows.ap[0]), [0, HD], [1, w]],
                )
                nc.sync.dma_start(out=dst, in_=rep)
                pbc_tiles[(c, i)] = dst

        acc = {}

        def emit_pv(c):
            """PV multiplies: Pool-share first (ready earliest — its bcasts
            were issued first), then DVE-share; identity-matmul accumulation
            follows each multiply on PE."""
            b = C * c
            order = pv_order(c)
            valid_g = [
                [i for i in order if vstart(c, i) <= 512 * gl] for gl in range(2)
            ]
            for gl in range(2):
                acc[2 * c + gl] = ps_a.tile(
                    [P, 512], fp32, tag="acc", name=f"acc_{2 * c + gl}"
                )
            started = [False, False]
            remaining = [len(valid_g[0]), len(valid_g[1])]
            for i in order:
                v5 = vstart(c, i)
                d = ALL_OFFSETS[i]
                w = C - v5
                tmp = tmps.tile([P, w], bf16, tag="tmp", name=f"tmp_{c}_{i}")
                eng = nc.gpsimd if (c, i) in POOL_PV else nc.vector
                eng.tensor_tensor(
                    out=tmp,
                    in0=pbc_tiles[(c, i)],
                    in1=vT[:, PAD + b + v5 - d : PAD + b + v5 - d + w],
                    op=MULT,
                )
                for gl in range(2):
                    if i not in valid_g[gl]:
                        continue
                    s0l = 512 * gl
                    remaining[gl] -= 1
                    nc.tensor.matmul(
                        acc[2 * c + gl], ident_w,
                        tmp[:, s0l - v5 : s0l - v5 + 512],
                        start=(not started[gl]),
                        stop=(remaining[gl] == 0),
                        skip_group_check=True,
                    )
                    started[gl] = True

        def emit_out(c):
            # per-half copy + DMA so the first half's output ships while the
            # second half is still copying (shortens the tail)
            for gl in range(2):
                g = 2 * c + gl
                s0 = 512 * g
                nc.scalar.activation(
                    out=out_sb[:, s0 : s0 + 512], in_=acc[g], func=COPY
                )
                nc.scalar.dma_start(
                    out=oT_out[:, s0 : s0 + 512], in_=out_sb[:, s0 : s0 + 512]
                )

        def emit_lcopy(pair):
            nc.scalar.activation(
                out=l_sb[:, 512 * pair : 512 * pair + 512],
                in_=psl8[pair], func=COPY,
            )

        # ---- pipelined emission: DVE runs products c0,c1,c2, squeezes in
        # pv(c0) while bcasts for c1/c2 land, then products c3 and the
        # remaining pv windows. Pool front-loads products (c1,c2,c3) then
        # drains its pv shares. ----
        emit_scores(0)
        emit_pbc(0)
        for cc in (2, 3):
            for i in POOL_PRODS[cc]:
                emit_product(cc, i, nc.gpsimd)
        emit_scores(1)
        emit_pbc(1)
        emit_scores(2)
        emit_pbc(2)
        emit_psl(0)
        emit_pv(0)
        emit_out(0)
        emit_scores(3)
        emit_pbc(3)
        emit_psl(1)
        emit_lcopy(0)                    # psl pair 0 complete after psl(c1)
        emit_pv(1)
        emit_out(1)
        emit_psl(2)
        emit_pv(2)
        emit_out(2)
        emit_psl(3)
        emit_lcopy(1)
        emit_pv(3)
        emit_out(3)
        nc.scalar.dma_start(out=l_out[:, :], in_=l_sb)

        bcast.release()
        tmps.release()
        work.release()
        ps_a.release()
        ps_l.release()
        ps_s.release()
        big.release()
        consts.release()

    nc.compile()
    return nc


def _prep_inputs(q, k, v, pos_bias):
    """Host-side sharding + layout prep: per core, heads (2c, 2c+1) packed as
    128 partitions (h*64+d), transposed to [dh, pad+n] bf16."""
    def to_T(x):
        xt = np.ascontiguousarray(x[0].transpose(0, 2, 1)).astype(BF16)
        xt = xt.reshape(8, P, N)
        return np.concatenate([np.zeros((8, P, PAD), dtype=BF16), xt], axis=2)

    qT = to_T(q)
    kT = to_T(k)
    vT = to_T(v)

    in_maps = []
    for c in range(8):
        pb = np.zeros((2 * NOFF, 1), dtype=np.float32)
        for i in range(NOFF):
            for hh in range(2):
                pb[2 * i + hh, 0] = pos_bias[i, 2 * c + hh]
        in_maps.append(
            {
                "qT_in": qT[c],
                "kT_in": kT[c],
                "vT_in": vT[c],
                "pb_in": pb.astype(BF16),
            }
        )
    return in_maps


def kernel(q, k, v, pos_bias, scale_embed):
    from concourse.bass_utils import run_bass_kernel_spmd

    q = np.asarray(q)
    k = np.asarray(k)
    v = np.asarray(v)
    pos_bias = np.asarray(pos_bias)
    scale_embed = np.asarray(scale_embed)
    assert q.shape == (1, 16, N, HD)

    key = scale_embed.tobytes()
    if key not in _CACHE:
        _CACHE.clear()
        _CACHE[key] = _build(scale_embed)
    nc = _CACHE[key]

    in_maps = _prep_inputs(q, k, v, pos_bias)
    res = run_bass_kernel_spmd(nc, in_maps, core_ids=list(range(8)), trace=TRACE)
    LAST_RESULTS[0] = res
    out = np.zeros((1, 16, N, HD), dtype=np.float32)
    for c in range(8):
        oT = res.results[c]["oT_out"]          # [128, N] bf16 (unnormalized)
        lv = res.results[c]["l_out"]           # [8, 1024] fp32
        # l[h, n]: n = 2048*pair + 512*hf + j -> l_out[2*hf+h, 512*pair+j]
        l = np.zeros((2, N), dtype=np.float32)
        for h in range(2):
            for pair in range(2):
                for hf in range(4):
                    n0 = 2048 * pair + 512 * hf
                    l[h, n0 : n0 + 512] = lv[2 * hf + h, 512 * pair : 512 * pair + 512]
        l = np.where(l > 0.0, l, 1.0)
        o = oT.astype(np.float32).reshape(2, HD, N).transpose(0, 2, 1)  # [2, N, HD]
        o = o / l[:, :, None]
        o[:, 0, :] = 0.0
        out[0, 2 * c : 2 * c + 2] = o
    return out
